# revision 13
# baseline (speedup 1.0000x reference)
"""LSTM decoder (teacher_forcing_ratio=0) on 8 TRN2 NeuronCores.

Strategy (v2): DP4 x TP2, "h-stationary" matmuls, pairwise exchange.
----------------------------------------------------------------------
Cores are grouped in 4 pairs; pair g owns batch block g (128 of 512
columns).  Within a pair, the 8192 gate rows are split in half (core
parity p takes hidden rows [p*1024:(p+1)*1024] of each of the i,f,g,o
blocks).  The autoregressive feedback x_{t+1} = Linear(h_t) is folded
into the recurrence (W_eff = W_hh + W_ih @ W_out), so each step is

    gates_t^T[b, gc] = sum_k hT_{t-1}[k, b] * W_eff^T[k, gc]

computed with h^T tiles as the PE *stationary* operand and W_eff^T as
the *moving* operand -- weight (h-tile) loads are then negligible (16
per step instead of 128+).  The batch block (128) is exactly the PSUM
partition width; the 4096 gate columns per core form 8 PSUM banks of
512 fp32.

Per-step cross-core traffic is only the pair exchange of the partner's
half of h_t (256 KB fp16) via a 2-rank AllGather, which overlaps the
"own half" (phase L) matmuls of the next step.  SPMD parity asymmetry
(which AllGather section holds the partner) is resolved with
partition-id-conditional DMAs (dma_start cond=) reading the partner
section of the gather output.

All matmul operands are fp16 (10-bit mantissa, ~tf32 accuracy); the
cell state c stays fp32 in SBUF.  Only tgt[:, 0] is consumed by the
reference, so just that frame is shipped.
"""

import os

import numpy as np

B, T_FULL, D, H = 512, 128, 128, 2048
NCORES = 8
NPAIR = 4
GC = 4096            # gate columns per core (8192 / 2)
NCH = 8              # PSUM chunks of 512 gate columns
KT = 16              # 128-row k-tiles of the hidden dim
KH = 8               # k-tiles per half

_CACHE = {}


def _build(t_steps, with_bias=False):
    import concourse.bacc as bacc
    import concourse.mybir as mybir
    from concourse import tile

    f32 = mybir.dt.float32
    f16 = mybir.dt.float16
    AF = mybir.ActivationFunctionType

    nc = bacc.Bacc("TRN2", target_bir_lowering=False, debug=False,
                   num_devices=NCORES)

    w_eff = nc.dram_tensor("w_eff", [128, KT * GC], f16, kind="ExternalInput")
    w_ih = nc.dram_tensor("w_ih", [128, GC], f16, kind="ExternalInput")
    w_out = nc.dram_tensor("w_out", [128, KT * 64], f16, kind="ExternalInput")
    x0t = nc.dram_tensor("x0t", [128, 128], f16, kind="ExternalInput")
    ident = nc.dram_tensor("ident", [128, 128], f16, kind="ExternalInput")
    if with_bias:
        befft = nc.dram_tensor("befft", [1, GC], f16, kind="ExternalInput")
        b0t = nc.dram_tensor("b0t", [1, GC], f16, kind="ExternalInput")
        boutt = nc.dram_tensor("boutt", [1, 64], f16, kind="ExternalInput")
        onesb = nc.dram_tensor("onesb", [1, 128], f16, kind="ExternalInput")
    out_d = nc.dram_tensor("out", [t_steps, 128, 64], f32,
                           kind="ExternalOutput")
    inb = [nc.dram_tensor(f"inb{i}", [1024, 128], f16) for i in range(2)]
    outb = [nc.dram_tensor(f"outb{i}", [2048, 128], f16) for i in range(2)]

    rg = [[2 * g, 2 * g + 1] for g in range(NPAIR)]
    R_ORDER = [0, 2, 4, 6, 1, 3, 5, 7]

    with tile.TileContext(nc) as tc:
        with (
            tc.tile_pool(name="w", bufs=1) as wp,
            tc.tile_pool(name="st", bufs=1) as stp,
            tc.tile_pool(name="sp", bufs=2) as sp,
            tc.tile_pool(name="ot", bufs=3) as otp,
            tc.tile_pool(name="ps", bufs=8, space="PSUM") as ps,
        ):
            w_eff_sb = wp.tile([128, KT * GC], f16)
            w_ih_sb = wp.tile([128, GC], f16)
            w_out_sb = wp.tile([128, KT * 64], f16)
            x0_sb = wp.tile([128, 128], f16)
            ident_sb = wp.tile([128, 128], f16)
            c_state = stp.tile([128, 1024], f32)

            nc.sync.dma_start(w_eff_sb[:], w_eff[:])
            nc.sync.dma_start(w_ih_sb[:], w_ih[:])
            nc.sync.dma_start(w_out_sb[:], w_out[:])
            nc.sync.dma_start(x0_sb[:], x0t[:])
            nc.sync.dma_start(ident_sb[:], ident[:])
            if with_bias:
                beff_sb = wp.tile([1, GC], f16)
                b0_sb = wp.tile([1, GC], f16)
                bout_sb = wp.tile([1, 64], f16)
                ones_sb = wp.tile([1, 128], f16)
                nc.sync.dma_start(beff_sb[:], befft[:])
                nc.sync.dma_start(b0_sb[:], b0t[:])
                nc.sync.dma_start(bout_sb[:], boutt[:])
                nc.sync.dma_start(ones_sb[:], onesb[:])

            # parity of this core: selects which AllGather section holds
            # the partner's h-half (even core -> partner is rank1).
            parity = nc.sync.partition_id() % 2

            def assemble_partner(t):
                """prt <- the partner half of outb, by conditional DMA.

                Split in two k-halves so phase R can start on k-tiles
                8..11 while 12..15 are still landing.
                """
                prt = sp.tile([128, 1024], f16, name="prt", tag="prt")
                for half in range(2):
                    sl = slice(half * 512, (half + 1) * 512)
                    for sec in range(2):
                        base = sec * 1024 + half * 512
                        nc.sync.dma_start(
                            prt[:, sl].rearrange("p (k n) -> p k n", k=4),
                            outb[t % 2].ap()[base:base + 512]
                            .rearrange("(k p) n -> p k n", k=4),
                            cond=(parity == 1 - sec))
                return prt

            def emit_step(t, stag_prev, prt):
                """Gates + interleaved update for step t; returns h16."""
                chs = [ps.tile([128, 512], f32, name=f"ch{c}", tag="bank")
                       for c in range(NCH)]
                sig = sp.tile([128, GC], f32, name="sig", tag="sig", bufs=1)
                h16 = sp.tile([128, 1024], f16, name="h16", tag="h16")
                tnc = sp.tile([128, 1024], f32, name="tnc", tag="tnc")
                tmp = sp.tile([128, 1024], f32, name="tmp", tag="tmp")

                def act(c):
                    func = AF.Tanh if c // 2 == 2 else AF.Sigmoid
                    nc.scalar.activation(sig[:, c * 512:(c + 1) * 512],
                                         chs[c][:], func)

                if t == 0:
                    for c in range(NCH):
                        nc.tensor.matmul(chs[c][:], x0_sb[:],
                                         w_ih_sb[:, c * 512:(c + 1) * 512],
                                         start=True, stop=not with_bias)
                        if with_bias:
                            nc.tensor.matmul(chs[c][:], ones_sb[0:1, 0:128],
                                             b0_sb[0:1, c * 512:(c + 1) * 512],
                                             start=False, stop=True)
                        act(c)
                    for s in range(2):
                        upd_cs(t, sig, tnc, tmp, s)
                        upd_h(sig, tnc, h16, s)
                else:
                    # phase L: own half of h (stag_prev), k-tiles 0..7
                    for kl in range(KH):
                        st = stag_prev[:, kl * 128:(kl + 1) * 128]
                        for c in range(NCH):
                            nc.tensor.matmul(
                                chs[c][:], st,
                                w_eff_sb[:, kl * GC + c * 512:
                                         kl * GC + (c + 1) * 512],
                                start=(kl == 0), stop=False)
                    # phase R: partner half, k-tiles 8..15, staggered chunks.
                    # The update is emitted in lockstep so the scalar/vector
                    # FIFOs see tanh(c) work before the last activation.
                    for idx, c in enumerate(R_ORDER):
                        for kr in range(KH):
                            nc.tensor.matmul(
                                chs[c][:], prt[:, kr * 128:(kr + 1) * 128],
                                w_eff_sb[:, (KH + kr) * GC + c * 512:
                                         (KH + kr) * GC + (c + 1) * 512],
                                start=False,
                                stop=(kr == KH - 1 and not with_bias))
                        if with_bias:
                            nc.tensor.matmul(
                                chs[c][:], ones_sb[0:1, 0:128],
                                beff_sb[0:1, c * 512:(c + 1) * 512],
                                start=False, stop=True)
                        act(c)
                        if idx == 3:      # i,f,g,o of slice A done
                            upd_cs(t, sig, tnc, tmp, 0)
                            upd_h(sig, tnc, h16, 0)
                        elif idx == 6:    # i,f,g of slice B done
                            upd_cs(t, sig, tnc, tmp, 1)
                        elif idx == 7:    # o of slice B done
                            upd_h(sig, tnc, h16, 1)
                return h16

            def upd_cs(t, sig, tnc, tmp, s):
                """Vector c-update + tanh(c) for h-slice s (512 cols)."""
                sl = slice(s * 512, (s + 1) * 512)
                si = sig[:, 0 * 1024 + s * 512:0 * 1024 + s * 512 + 512]
                sf = sig[:, 1 * 1024 + s * 512:1 * 1024 + s * 512 + 512]
                tg = sig[:, 2 * 1024 + s * 512:2 * 1024 + s * 512 + 512]
                cs = c_state[:, sl]
                if t == 0:
                    nc.vector.tensor_mul(cs, si, tg)
                else:
                    nc.vector.tensor_mul(cs, sf, cs)
                    nc.vector.tensor_mul(tmp[:, sl], si, tg)
                    nc.vector.tensor_add(cs, cs, tmp[:, sl])
                nc.scalar.activation(tnc[:, sl], cs, AF.Tanh)

            def upd_h(sig, tnc, h16, s):
                sl = slice(s * 512, (s + 1) * 512)
                so = sig[:, 3 * 1024 + s * 512:3 * 1024 + s * 512 + 512]
                nc.vector.tensor_mul(h16[:, sl], so, tnc[:, sl])

            def emit_transpose_half(t, h16, stag, half):
                for kk in range(half * 4, half * 4 + 4):
                    tp = ps.tile([128, 128], f16, name=f"tp{kk}", tag="bank")
                    nc.tensor.transpose(tp[:],
                                        h16[:, kk * 128:(kk + 1) * 128],
                                        ident_sb[:])
                    nc.vector.tensor_copy(
                        stag[:, kk * 128:(kk + 1) * 128], tp[:])
                # ship each half as soon as its transposes land
                nc.sync.dma_start(
                    inb[t % 2].ap()[half * 512:(half + 1) * 512]
                    .rearrange("(k p) n -> p k n", k=4),
                    stag[:, half * 512:(half + 1) * 512]
                    .rearrange("p (k n) -> p k n", k=4))

            def emit_wout(t, stag_prev, prt):
                po = ps.tile([128, 64], f32, name="po", tag="bank")
                for kk in range(KH):
                    nc.tensor.matmul(po[:], stag_prev[:, kk * 128:(kk + 1) * 128],
                                     w_out_sb[:, kk * 64:(kk + 1) * 64],
                                     start=(kk == 0), stop=False)
                for kr in range(KH):
                    nc.tensor.matmul(po[:], prt[:, kr * 128:(kr + 1) * 128],
                                     w_out_sb[:, (KH + kr) * 64:
                                              (KH + kr + 1) * 64],
                                     start=False,
                                     stop=(kr == KH - 1 and not with_bias))
                if with_bias:
                    nc.tensor.matmul(po[:], ones_sb[0:1, 0:128],
                                     bout_sb[0:1, 0:64],
                                     start=False, stop=True)
                ot = otp.tile([128, 64], f32, name="ot", tag="ot")
                nc.scalar.copy(ot[:], po[:])
                nc.sync.dma_start(out_d[t], ot[:])

            stag_prev = None
            prt = None
            for t in range(t_steps):
                if t > 0:
                    prt = assemble_partner(t - 1)
                h16 = emit_step(t, stag_prev, prt)
                stag = sp.tile([128, 1024], f16, name="stag", tag="stag")
                # PE order: transposes-A, wout (covers the h16-B wait),
                # transposes-B; each inb half ships as soon as it lands.
                emit_transpose_half(t, h16, stag, 0)
                if t > 0:
                    emit_wout(t - 1, stag_prev, prt)
                emit_transpose_half(t, h16, stag, 1)
                if os.environ.get("LSTM_NOEX", "0") != "1":
                    nc.gpsimd.collective_compute(
                        "AllGather", mybir.AluOpType.bypass,
                        replica_groups=rg,
                        ins=[inb[t % 2].ap().opt()],
                        outs=[outb[t % 2].ap().opt()],
                    )
                stag_prev = stag
            prt = assemble_partner(t_steps - 1)
            emit_wout(t_steps - 1, stag_prev, prt)

    nc.compile()
    return nc


_build_pipe = _build


def _prep_inputs(tgt, W_ih, W_hh, b_ih, b_hh, W_out, b_out, t_steps):
    f32 = np.float32
    f16 = np.float16
    tgt = np.asarray(tgt, f32)
    W_ih = np.asarray(W_ih, f32)
    W_hh = np.asarray(W_hh, f32)
    W_out = np.asarray(W_out, f32)
    b = np.asarray(b_ih, f32) + np.asarray(b_hh, f32)
    b_out = np.asarray(b_out, f32)

    W_eff = W_hh + W_ih @ W_out          # [4H, H]
    b_eff = b + W_ih @ b_out             # [4H]
    with_bias = bool(np.any(b) or np.any(b_eff) or np.any(b_out))

    x0 = tgt[:, 0, :]                    # [B, D]
    ident = np.eye(128, dtype=f16)

    in_maps = []
    for core in range(NCORES):
        g, p = core // 2, core % 2
        rows_sel = np.concatenate(
            [G * H + p * 1024 + np.arange(1024) for G in range(4)])
        perm = np.concatenate([p * 1024 + np.arange(1024),
                               (1 - p) * 1024 + np.arange(1024)])
        WT = W_eff[rows_sel].T[perm]     # [2048(h, own-first), 4096]
        w_eff_arr = np.ascontiguousarray(
            WT.reshape(KT, 128, GC).transpose(1, 0, 2)
            .reshape(128, KT * GC).astype(f16))
        w_ih_arr = np.ascontiguousarray(W_ih[rows_sel].T.astype(f16))
        WoT = W_out[p * 64:(p + 1) * 64, :].T[perm]   # [2048, 64]
        w_out_arr = np.ascontiguousarray(
            WoT.reshape(KT, 128, 64).transpose(1, 0, 2)
            .reshape(128, KT * 64).astype(f16))
        x0t_arr = np.ascontiguousarray(
            x0[g * 128:(g + 1) * 128, :].T.astype(f16))
        m = {"w_eff": w_eff_arr, "w_ih": w_ih_arr, "w_out": w_out_arr,
             "x0t": x0t_arr, "ident": ident}
        if with_bias:
            m["befft"] = np.ascontiguousarray(
                b_eff[rows_sel][None].astype(f16))
            m["b0t"] = np.ascontiguousarray(b[rows_sel][None].astype(f16))
            m["boutt"] = np.ascontiguousarray(
                b_out[p * 64:(p + 1) * 64][None].astype(f16))
            m["onesb"] = np.ones((1, 128), f16)
        in_maps.append(m)
    return in_maps


def kernel(tgt, W_ih, W_hh, b_ih, b_hh, W_out, b_out):
    from concourse.bass_utils import run_bass_kernel_spmd

    t_steps = int(os.environ.get("LSTM_T", T_FULL))
    pipe = os.environ.get("LSTM_PIPE", "1") == "1"

    b = np.asarray(b_ih, np.float32) + np.asarray(b_hh, np.float32)
    b_eff = b + np.asarray(W_ih, np.float32) @ np.asarray(b_out, np.float32)
    with_bias = bool(np.any(b) or np.any(b_eff) or np.any(b_out))

    key = (t_steps, pipe)
    if key not in _CACHE:
        _CACHE[key] = _build(t_steps, with_bias=with_bias)
    nc = _CACHE[key]

    in_maps = _prep_inputs(tgt, W_ih, W_hh, b_ih, b_hh, W_out, b_out, t_steps)
    res = run_bass_kernel_spmd(nc, in_maps, core_ids=list(range(NCORES)))

    full = np.empty((B, t_steps, D), np.float32)
    for core in range(NCORES):
        g, p = core // 2, core % 2
        o = res.results[core]["out"]               # [t, 128, 64]
        full[g * 128:(g + 1) * 128, :, p * 64:(p + 1) * 64] = \
            o.transpose(1, 0, 2)
    return full


# revision 18
# speedup vs baseline: 1.1031x; 1.1031x over previous
"""LSTM decoder (teacher_forcing_ratio=0) on 8 TRN2 NeuronCores.

Strategy (v2): DP4 x TP2, "h-stationary" matmuls, pairwise exchange.
----------------------------------------------------------------------
Cores are grouped in 4 pairs; pair g owns batch block g (128 of 512
columns).  Within a pair, the 8192 gate rows are split in half (core
parity p takes hidden rows [p*1024:(p+1)*1024] of each of the i,f,g,o
blocks).  The autoregressive feedback x_{t+1} = Linear(h_t) is folded
into the recurrence (W_eff = W_hh + W_ih @ W_out), so each step is

    gates_t^T[b, gc] = sum_k hT_{t-1}[k, b] * W_eff^T[k, gc]

computed with h^T tiles as the PE *stationary* operand and W_eff^T as
the *moving* operand -- weight (h-tile) loads are then negligible (16
per step instead of 128+).  The batch block (128) is exactly the PSUM
partition width; the 4096 gate columns per core form 8 PSUM banks of
512 fp32.

Per-step cross-core traffic is only the pair exchange of the partner's
half of h_t (256 KB fp16) via a 2-rank AllGather, which overlaps the
"own half" (phase L) matmuls of the next step.  SPMD parity asymmetry
(which AllGather section holds the partner) is resolved with
partition-id-conditional DMAs (dma_start cond=) reading the partner
section of the gather output.

All matmul operands are fp16 (10-bit mantissa, ~tf32 accuracy); the
cell state c stays fp32 in SBUF.  Only tgt[:, 0] is consumed by the
reference, so just that frame is shipped.
"""

import os

import numpy as np

B, T_FULL, D, H = 512, 128, 128, 2048
NCORES = 8
NPAIR = 4
GC = 4096            # gate columns per core (8192 / 2)
NCH = 8              # PSUM chunks of 512 gate columns
KT = 16              # 128-row k-tiles of the hidden dim
KH = 8               # k-tiles per half

_CACHE = {}


def _build(t_steps, with_bias=False):
    import concourse.bacc as bacc
    import concourse.mybir as mybir
    from concourse import tile

    f32 = mybir.dt.float32
    f16 = mybir.dt.float16
    AF = mybir.ActivationFunctionType

    nc = bacc.Bacc("TRN2", target_bir_lowering=False, debug=False,
                   num_devices=NCORES)

    w_eff = nc.dram_tensor("w_eff", [128, KT * GC], f16, kind="ExternalInput")
    w_ih = nc.dram_tensor("w_ih", [128, GC], f16, kind="ExternalInput")
    w_out = nc.dram_tensor("w_out", [128, KT * 64], f16, kind="ExternalInput")
    x0t = nc.dram_tensor("x0t", [128, 128], f16, kind="ExternalInput")
    ident = nc.dram_tensor("ident", [128, 128], f16, kind="ExternalInput")
    if with_bias:
        befft = nc.dram_tensor("befft", [1, GC], f16, kind="ExternalInput")
        b0t = nc.dram_tensor("b0t", [1, GC], f16, kind="ExternalInput")
        boutt = nc.dram_tensor("boutt", [1, 64], f16, kind="ExternalInput")
        onesb = nc.dram_tensor("onesb", [1, 128], f16, kind="ExternalInput")
    out_d = nc.dram_tensor("out", [t_steps, 128, 64], f32,
                           kind="ExternalOutput")
    inb = [nc.dram_tensor(f"inb{i}", [1024, 128], f16) for i in range(2)]
    outb = [nc.dram_tensor(f"outb{i}", [2048, 128], f16) for i in range(2)]

    rg = [[2 * g, 2 * g + 1] for g in range(NPAIR)]
    R_ORDER = [0, 2, 4, 6, 1, 3, 5, 7]

    with tile.TileContext(nc) as tc:
        with (
            tc.tile_pool(name="w", bufs=1) as wp,
            tc.tile_pool(name="st", bufs=1) as stp,
            tc.tile_pool(name="sp", bufs=2) as sp,
            tc.tile_pool(name="ot", bufs=3) as otp,
            tc.tile_pool(name="ps", bufs=8, space="PSUM") as ps,
        ):
            w_eff_sb = wp.tile([128, KT * GC], f16)
            w_ih_sb = wp.tile([128, GC], f16)
            w_out_sb = wp.tile([128, KT * 64], f16)
            x0_sb = wp.tile([128, 128], f16)
            ident_sb = wp.tile([128, 128], f16)
            c_state = stp.tile([128, 1024], f32)

            nc.sync.dma_start(w_eff_sb[:], w_eff[:])
            nc.sync.dma_start(w_ih_sb[:], w_ih[:])
            nc.sync.dma_start(w_out_sb[:], w_out[:])
            nc.sync.dma_start(x0_sb[:], x0t[:])
            nc.sync.dma_start(ident_sb[:], ident[:])
            if with_bias:
                beff_sb = wp.tile([1, GC], f16)
                b0_sb = wp.tile([1, GC], f16)
                bout_sb = wp.tile([1, 64], f16)
                ones_sb = wp.tile([1, 128], f16)
                nc.sync.dma_start(beff_sb[:], befft[:])
                nc.sync.dma_start(b0_sb[:], b0t[:])
                nc.sync.dma_start(bout_sb[:], boutt[:])
                nc.sync.dma_start(ones_sb[:], onesb[:])

            # parity of this core: selects which AllGather section holds
            # the partner's h-half (even core -> partner is rank1).
            parity = nc.sync.partition_id() % 2

            def assemble_partner(t):
                """prt <- the partner half of outb, by conditional DMA.

                Split in two k-halves so phase R can start on k-tiles
                8..11 while 12..15 are still landing.
                """
                prt = sp.tile([128, 1024], f16, name="prt", tag="prt")
                for half in range(2):
                    sl = slice(half * 512, (half + 1) * 512)
                    for sec in range(2):
                        base = sec * 1024 + half * 512
                        nc.sync.dma_start(
                            prt[:, sl].rearrange("p (k n) -> p k n", k=4),
                            outb[t % 2].ap()[base:base + 512]
                            .rearrange("(k p) n -> p k n", k=4),
                            cond=(parity == 1 - sec))
                return prt

            def emit_step(t, stag_prev, prt):
                """Gates + interleaved update for step t; returns h16."""
                chs = [ps.tile([128, 512], f32, name=f"ch{c}", tag="bank")
                       for c in range(NCH)]
                sig = sp.tile([128, GC], f32, name="sig", tag="sig", bufs=1)
                h16 = sp.tile([128, 1024], f16, name="h16", tag="h16")
                tnc = sp.tile([128, 1024], f32, name="tnc", tag="tnc")
                tmp = sp.tile([128, 1024], f32, name="tmp", tag="tmp")

                def act(c):
                    func = AF.Tanh if c // 2 == 2 else AF.Sigmoid
                    nc.scalar.activation(sig[:, c * 512:(c + 1) * 512],
                                         chs[c][:], func)

                if t == 0:
                    for c in range(NCH):
                        nc.tensor.matmul(chs[c][:], x0_sb[:],
                                         w_ih_sb[:, c * 512:(c + 1) * 512],
                                         start=True, stop=not with_bias)
                        if with_bias:
                            nc.tensor.matmul(chs[c][:], ones_sb[0:1, 0:128],
                                             b0_sb[0:1, c * 512:(c + 1) * 512],
                                             start=False, stop=True)
                        act(c)
                    for s in range(2):
                        upd_cs(t, sig, tnc, tmp, s)
                        upd_h(sig, tnc, h16, s)
                else:
                    # phase L: own half of h (stag_prev), k-tiles 0..7
                    for kl in range(KH):
                        st = stag_prev[:, kl * 128:(kl + 1) * 128]
                        for c in range(NCH):
                            nc.tensor.matmul(
                                chs[c][:], st,
                                w_eff_sb[:, kl * GC + c * 512:
                                         kl * GC + (c + 1) * 512],
                                start=(kl == 0), stop=False)
                    # phase R: partner half, k-tiles 8..15, staggered chunks.
                    # The update is emitted in lockstep so the scalar/vector
                    # FIFOs see tanh(c) work before the last activation.
                    for idx, c in enumerate(R_ORDER):
                        for kr in range(KH):
                            nc.tensor.matmul(
                                chs[c][:], prt[:, kr * 128:(kr + 1) * 128],
                                w_eff_sb[:, (KH + kr) * GC + c * 512:
                                         (KH + kr) * GC + (c + 1) * 512],
                                start=False,
                                stop=(kr == KH - 1 and not with_bias))
                        if with_bias:
                            nc.tensor.matmul(
                                chs[c][:], ones_sb[0:1, 0:128],
                                beff_sb[0:1, c * 512:(c + 1) * 512],
                                start=False, stop=True)
                        act(c)
                        if idx == 3:      # i,f,g,o of slice A done
                            upd_cs(t, sig, tnc, tmp, 0)
                            upd_h(sig, tnc, h16, 0)
                        elif idx == 6:    # i,f,g of slice B done
                            upd_cs(t, sig, tnc, tmp, 1)
                        elif idx == 7:    # o of slice B done
                            upd_h(sig, tnc, h16, 1)
                return h16

            def upd_cs(t, sig, tnc, tmp, s):
                """Vector c-update + tanh(c) for h-slice s (512 cols)."""
                sl = slice(s * 512, (s + 1) * 512)
                si = sig[:, 0 * 1024 + s * 512:0 * 1024 + s * 512 + 512]
                sf = sig[:, 1 * 1024 + s * 512:1 * 1024 + s * 512 + 512]
                tg = sig[:, 2 * 1024 + s * 512:2 * 1024 + s * 512 + 512]
                cs = c_state[:, sl]
                if t == 0:
                    nc.vector.tensor_mul(cs, si, tg)
                else:
                    nc.vector.tensor_mul(cs, sf, cs)
                    nc.vector.tensor_mul(tmp[:, sl], si, tg)
                    nc.vector.tensor_add(cs, cs, tmp[:, sl])
                nc.scalar.activation(tnc[:, sl], cs, AF.Tanh)

            def upd_h(sig, tnc, h16, s):
                sl = slice(s * 512, (s + 1) * 512)
                so = sig[:, 3 * 1024 + s * 512:3 * 1024 + s * 512 + 512]
                nc.vector.tensor_mul(h16[:, sl], so, tnc[:, sl])

            def emit_transpose_half(t, h16, stag, half):
                for kk in range(half * 4, half * 4 + 4):
                    tp = ps.tile([128, 128], f16, name=f"tp{kk}", tag="bank")
                    nc.tensor.transpose(tp[:],
                                        h16[:, kk * 128:(kk + 1) * 128],
                                        ident_sb[:])
                    nc.vector.tensor_copy(
                        stag[:, kk * 128:(kk + 1) * 128], tp[:])
                # ship each half as soon as its transposes land
                nc.sync.dma_start(
                    inb[t % 2].ap()[half * 512:(half + 1) * 512]
                    .rearrange("(k p) n -> p k n", k=4),
                    stag[:, half * 512:(half + 1) * 512]
                    .rearrange("p (k n) -> p k n", k=4))

            def emit_wout(t, stag_prev, prt):
                po = ps.tile([128, 64], f32, name="po", tag="bank")
                for kk in range(KH):
                    nc.tensor.matmul(po[:], stag_prev[:, kk * 128:(kk + 1) * 128],
                                     w_out_sb[:, kk * 64:(kk + 1) * 64],
                                     start=(kk == 0), stop=False)
                for kr in range(KH):
                    nc.tensor.matmul(po[:], prt[:, kr * 128:(kr + 1) * 128],
                                     w_out_sb[:, (KH + kr) * 64:
                                              (KH + kr + 1) * 64],
                                     start=False,
                                     stop=(kr == KH - 1 and not with_bias))
                if with_bias:
                    nc.tensor.matmul(po[:], ones_sb[0:1, 0:128],
                                     bout_sb[0:1, 0:64],
                                     start=False, stop=True)
                ot = otp.tile([128, 64], f32, name="ot", tag="ot")
                nc.scalar.copy(ot[:], po[:])
                nc.sync.dma_start(out_d[t], ot[:])

            stag_prev = None
            prt = None
            for t in range(t_steps):
                if t > 0:
                    prt = assemble_partner(t - 1)
                h16 = emit_step(t, stag_prev, prt)
                stag = sp.tile([128, 1024], f16, name="stag", tag="stag")
                # PE order: transposes-A, wout (covers the h16-B wait),
                # transposes-B; each inb half ships as soon as it lands.
                emit_transpose_half(t, h16, stag, 0)
                if t > 0:
                    emit_wout(t - 1, stag_prev, prt)
                emit_transpose_half(t, h16, stag, 1)
                if os.environ.get("LSTM_NOEX", "0") != "1":
                    nc.gpsimd.collective_compute(
                        "AllGather", mybir.AluOpType.bypass,
                        replica_groups=rg,
                        ins=[inb[t % 2].ap().opt()],
                        outs=[outb[t % 2].ap().opt()],
                    )
                stag_prev = stag
            prt = assemble_partner(t_steps - 1)
            emit_wout(t_steps - 1, stag_prev, prt)

    nc.compile()
    return nc


_build_pipe = _build


def _prep_inputs(tgt, W_ih, W_hh, b_ih, b_hh, W_out, b_out, t_steps):
    f32 = np.float32
    f16 = np.float16
    tgt = np.asarray(tgt, f32)
    W_ih = np.asarray(W_ih, f32)
    W_hh = np.asarray(W_hh, f32)
    W_out = np.asarray(W_out, f32)
    b = np.asarray(b_ih, f32) + np.asarray(b_hh, f32)
    b_out = np.asarray(b_out, f32)

    W_eff = W_hh + W_ih @ W_out          # [4H, H]
    b_eff = b + W_ih @ b_out             # [4H]
    with_bias = bool(np.any(b) or np.any(b_eff) or np.any(b_out))

    x0 = tgt[:, 0, :]                    # [B, D]
    ident = np.eye(128, dtype=f16)

    in_maps = []
    for core in range(NCORES):
        g, p = core // 2, core % 2
        rows_sel = np.concatenate(
            [G * H + p * 1024 + np.arange(1024) for G in range(4)])
        perm = np.concatenate([p * 1024 + np.arange(1024),
                               (1 - p) * 1024 + np.arange(1024)])
        WT = W_eff[rows_sel].T[perm]     # [2048(h, own-first), 4096]
        w_eff_arr = np.ascontiguousarray(
            WT.reshape(KT, 128, GC).transpose(1, 0, 2)
            .reshape(128, KT * GC).astype(f16))
        w_ih_arr = np.ascontiguousarray(W_ih[rows_sel].T.astype(f16))
        WoT = W_out[p * 64:(p + 1) * 64, :].T[perm]   # [2048, 64]
        w_out_arr = np.ascontiguousarray(
            WoT.reshape(KT, 128, 64).transpose(1, 0, 2)
            .reshape(128, KT * 64).astype(f16))
        x0t_arr = np.ascontiguousarray(
            x0[g * 128:(g + 1) * 128, :].T.astype(f16))
        m = {"w_eff": w_eff_arr, "w_ih": w_ih_arr, "w_out": w_out_arr,
             "x0t": x0t_arr, "ident": ident}
        if with_bias:
            m["befft"] = np.ascontiguousarray(
                b_eff[rows_sel][None].astype(f16))
            m["b0t"] = np.ascontiguousarray(b[rows_sel][None].astype(f16))
            m["boutt"] = np.ascontiguousarray(
                b_out[p * 64:(p + 1) * 64][None].astype(f16))
            m["onesb"] = np.ones((1, 128), f16)
        in_maps.append(m)
    return in_maps


def kernel(tgt, W_ih, W_hh, b_ih, b_hh, W_out, b_out):
    from concourse.bass_utils import run_bass_kernel_spmd

    t_steps = int(os.environ.get("LSTM_T", T_FULL))
    pipe = os.environ.get("LSTM_PIPE", "1") == "1"

    b = np.asarray(b_ih, np.float32) + np.asarray(b_hh, np.float32)
    b_eff = b + np.asarray(W_ih, np.float32) @ np.asarray(b_out, np.float32)
    with_bias = bool(np.any(b) or np.any(b_eff) or np.any(b_out))

    key = (t_steps, pipe)
    if key not in _CACHE:
        _CACHE[key] = _build(t_steps, with_bias=with_bias)
    nc = _CACHE[key]

    in_maps = _prep_inputs(tgt, W_ih, W_hh, b_ih, b_hh, W_out, b_out, t_steps)
    res = run_bass_kernel_spmd(nc, in_maps, core_ids=list(range(NCORES)))

    full = np.empty((B, t_steps, D), np.float32)
    for core in range(NCORES):
        g, p = core // 2, core % 2
        o = res.results[core]["out"]               # [t, 128, 64]
        full[g * 128:(g + 1) * 128, :, p * 64:(p + 1) * 64] = \
            o.transpose(1, 0, 2)
    return full
